# revision 9
# baseline (speedup 1.0000x reference)
"""ComplEx scoring kernel for Trainium2, sharded over 8 NeuronCores.

Computes: result[b, e] = tmp1[b] . E_im[e] + tmp2[b] . E_re[e] + mask[e]
where tmp1/tmp2 are complex-product combinations of gathered entity and
relation embeddings (with inverse-relation sign handling).

Sharding: entity dimension (100000) split across 8 cores (12500 each).
Batch and relation tables replicated. Each core redundantly computes the
gathered tmp1/tmp2 (needs the full entity table for the gather), then
GEMMs against its own entity shard and emits [1024, 12500] logits.

v3 (on top of v2's bf16 GEMM + HWDGE streaming + two-phase head order):
- gather tables in bf16; the inverse-relation sign and r_eff mapping are
  folded into a host-prepared 1000-row signed relation table, removing
  all on-device index preprocessing.
- tmp^T built by an SBUF-source transposing dma_gather (SWDGE) instead
  of 64 tensor-engine transposes + PSUM evacuations, so the PE runs
  matmuls only.
- mask tiles and output logits in bf16 (host converts back to f32).
"""

import sys

sys.path.insert(0, "/opt/trn_rl_repo")

import numpy as np

import concourse.bacc as bacc
import concourse.bass as bass
import concourse.mybir as mybir
import concourse.tile as tile
from concourse.bass import IndirectOffsetOnAxis
from concourse.bass_utils import run_bass_kernel_spmd

F32 = mybir.dt.float32
BF16 = mybir.dt.bfloat16
I32 = mybir.dt.int32
I16 = mybir.dt.int16

NUM_ENTITIES = 100000
DIM = 512
BATCH = 1024
NUM_REL_TOTAL = 1000
NUM_REL = 500  # NUM_REL_TOTAL // 2
N_CORES = 8
ESH = NUM_ENTITIES // N_CORES  # 12500 entities per core
ET = 500                       # entity tile (free dim of each matmul)
N_ET = ESH // ET               # 25 e-tiles per core
NB = BATCH // 128              # 8 batch tiles
KCH = 2 * DIM // 128           # 8 contraction chunks over [tmp1|tmp2]
NHEAD = 6                      # e-tiles kept resident for the two-phase head


def build_module(
    dtype="bf16",
    nrep=1,
    nhead=NHEAD,
    wgroup=2,           # e-tiles sharing one weight load
    out_bf16=True,      # store logits as bf16 (host converts back to f32)
    do_prologue=True,   # gathers + elementwise + transpose-gathers
    do_rhs_dma=True,    # stream rhs tiles from DRAM (else one static tile)
    do_mm=True,         # matmuls
    store_all=True,     # all output stores (else only et==0)
):
    assert dtype == "bf16"
    mm_dt = BF16
    nc = bacc.Bacc("TRN2", target_bir_lowering=False, debug=False)

    hix = nc.dram_tensor("hix", [128, NB], I32, kind="ExternalInput")
    rix = nc.dram_tensor("rix", [128, NB], I32, kind="ExternalInput")
    gidx = nc.dram_tensor("gidx", [128, KCH], I16, kind="ExternalInput")
    ecb = nc.dram_tensor("ecb", [NUM_ENTITIES, 2 * DIM], BF16, kind="ExternalInput")
    rpb = nc.dram_tensor("rpb", [NUM_REL_TOTAL, 2 * DIM], BF16, kind="ExternalInput")
    ecatT = nc.dram_tensor("ecatT", [128, N_ET * KCH * ET], mm_dt, kind="ExternalInput")
    maskb = nc.dram_tensor("maskb", [128, ESH], BF16, kind="ExternalInput")
    out_dt = BF16 if out_bf16 else F32
    out = nc.dram_tensor("out", [BATCH, ESH], out_dt, kind="ExternalOutput")

    # group e-tiles into units of `wgroup` that share each weight load
    units = [
        tuple(range(u, min(u + wgroup, N_ET))) for u in range(0, N_ET, wgroup)
    ]
    n_head_units = max(1, nhead // wgroup) if nhead else 0

    # rhs pool must keep the head units resident for the bt 4-7 revisit
    # (tiles are per-unit: wgroup e-tiles merged into one DMA/buffer)
    rhs_bufs = (n_head_units + 3) if nhead else 3
    psum_bufs = max(6, wgroup + 2)

    with tile.TileContext(nc) as tc:
        with (
            tc.tile_pool(name="cpool", bufs=1) as cpool,
            tc.tile_pool(name="gpool", bufs=3) as gpool,
            tc.tile_pool(name="epool", bufs=2) as epool,
            tc.tile_pool(name="persist", bufs=1) as ppool,
            tc.tile_pool(name="rhspool", bufs=rhs_bufs) as rhspool,
            tc.tile_pool(name="mpool", bufs=rhs_bufs) as mpool,
            tc.tile_pool(name="outpool", bufs=4) as outpool,
            tc.tile_pool(name="psum", bufs=psum_bufs, space="PSUM") as psum,
        ):
          for _rep in range(nrep):
            # ---- index tiles ----
            hix_sb = cpool.tile([128, NB], I32)
            nc.sync.dma_start(hix_sb[:], hix[:])
            rix_sb = cpool.tile([128, NB], I32)
            nc.sync.dma_start(rix_sb[:], rix[:])
            gidx_sb = cpool.tile([128, KCH], I16)
            nc.sync.dma_start(gidx_sb[:], gidx[:])

            # ---- gather + elementwise + transpose-gather: build tmpcatT ----
            # tmpcatT[bt][p, c, m] = tmpcat[bt*128 + m, c*128 + p]
            # where tmpcat = [tmp1 | tmp2] over the 1024-wide contraction.
            tmpcatT = [
                ppool.tile([128, KCH * 128], mm_dt, tag=f"tT{b}", name=f"tT{b}")
                for b in range(NB)
            ]

            if not do_prologue:
                scratch = cpool.tile([128, KCH * 128], F32)
                nc.vector.memset(scratch[:], 0.001)
                for b in range(NB):
                    nc.vector.tensor_copy(tmpcatT[b][:], scratch[:])
            for bt in range(NB if do_prologue else 0):
                # one gathered row of ecb/rpb = [im (512) | re (512)]
                g_e = gpool.tile([128, 2 * DIM], BF16, tag="g_e")
                nc.gpsimd.indirect_dma_start(
                    out=g_e[:], out_offset=None, in_=ecb[:],
                    in_offset=IndirectOffsetOnAxis(ap=hix_sb[:, bt : bt + 1], axis=0),
                )
                g_r = gpool.tile([128, 2 * DIM], BF16, tag="g_r")
                nc.gpsimd.indirect_dma_start(
                    out=g_r[:], out_offset=None, in_=rpb[:],
                    in_offset=IndirectOffsetOnAxis(ap=rix_sb[:, bt : bt + 1], axis=0),
                )
                h_im, h_re = g_e[:, :DIM], g_e[:, DIM:]
                rp_im, rp_re = g_r[:, :DIM], g_r[:, DIM:]

                tmpc = epool.tile([128, 2 * DIM], mm_dt, tag="tmpc")
                # tmp1 = h_im*rp_re + h_re*rp_im
                pa = epool.tile([128, DIM], F32, tag="pa")
                nc.vector.tensor_mul(pa[:], h_im, rp_re)
                pb = epool.tile([128, DIM], F32, tag="pb")
                nc.vector.tensor_mul(pb[:], h_re, rp_im)
                nc.vector.tensor_add(tmpc[:, :DIM], pa[:], pb[:])
                # tmp2 = h_re*rp_re - h_im*rp_im
                pc = epool.tile([128, DIM], F32, tag="pc")
                nc.vector.tensor_mul(pc[:], h_re, rp_re)
                pd = epool.tile([128, DIM], F32, tag="pd")
                nc.vector.tensor_mul(pd[:], h_im, rp_im)
                nc.vector.tensor_sub(tmpc[:, DIM:], pc[:], pd[:])

                # transpose via SBUF-source gather: one SWDGE op per bt
                nc.gpsimd.dma_gather(
                    out_ap=tmpcatT[bt][:].rearrange("p (a b) -> p a b", a=KCH),
                    in_ap=tmpc[:],
                    idxs_ap=gidx_sb[:],
                    num_idxs=128,
                    num_idxs_reg=128,
                    elem_size=2 * DIM,
                    transpose=True,
                    sbuf_tokens_per_rank=128,
                    sbuf_free_dim_per_rank=2 * DIM * 2,
                )

            # ---- main GEMM: out[b, e] = tmpcat @ Ecat^T + mask ----
            if not do_rhs_dma:
                rhs_static = rhspool.tile([128, wgroup * KCH * ET], mm_dt, tag="rhss")
                scratch2 = cpool.tile([128, wgroup * KCH * ET], F32)
                nc.vector.memset(scratch2[:], 0.001)
                nc.vector.tensor_copy(rhs_static[:], scratch2[:])

            if nhead:
                hu = units[:n_head_units]
                order = (
                    [(u, bt) for u in hu for bt in range(NB // 2)]
                    + [(u, bt) for u in hu for bt in range(NB // 2, NB)]
                    + [(u, bt) for u in units[n_head_units:] for bt in range(NB)]
                )
            else:
                order = [(u, bt) for u in units for bt in range(NB)]

            rhs_tiles, mask_tiles = {}, {}
            for unit, bt in order:
                uw = len(unit)
                e0 = unit[0]
                if unit not in rhs_tiles:
                    # one merged DMA per unit for rhs and mask (e-tiles in a
                    # unit are contiguous in both ecatT and out)
                    if do_rhs_dma:
                        r_t = rhspool.tile([128, wgroup * KCH * ET], mm_dt, tag="rhs")
                        nc.sync.dma_start(
                            r_t[:, : uw * KCH * ET],
                            ecatT[:, e0 * KCH * ET : (e0 + uw) * KCH * ET],
                        )
                        rhs_tiles[unit] = r_t
                    else:
                        rhs_tiles[unit] = rhs_static
                    m_t = mpool.tile([128, wgroup * ET], BF16, tag="mtile")
                    nc.sync.dma_start(
                        m_t[:, : uw * ET], maskb[:, e0 * ET : (e0 + uw) * ET]
                    )
                    mask_tiles[unit] = m_t

                if not do_mm:
                    continue
                pss = [
                    psum.tile([128, ET], F32, tag="ps", name=f"ps{i}")
                    for i in range(uw)
                ]
                for c in range(KCH):
                    w = tmpcatT[bt][:, c * 128 : (c + 1) * 128]
                    for i in range(uw):
                        nc.tensor.matmul(
                            pss[i][:],
                            lhsT=w,
                            rhs=rhs_tiles[unit][:, (i * KCH + c) * ET : (i * KCH + c + 1) * ET],
                            start=(c == 0),
                            stop=(c == KCH - 1),
                        )
                if store_all or e0 == 0:
                    ot = outpool.tile([128, wgroup * ET], out_dt, tag="ot")
                    for i in range(uw):
                        nc.vector.tensor_add(
                            ot[:, i * ET : (i + 1) * ET],
                            pss[i][:],
                            mask_tiles[unit][:, i * ET : (i + 1) * ET],
                        )
                    nc.sync.dma_start(
                        out[bt * 128 : (bt + 1) * 128, e0 * ET : (e0 + uw) * ET],
                        ot[:, : uw * ET],
                    )

    nc.compile()
    return nc


_NC_CACHE = {}


def _get_module(dtype="bf16"):
    if dtype not in _NC_CACHE:
        _NC_CACHE[dtype] = build_module(dtype)
    return _NC_CACHE[dtype]


def make_in_maps(h, r, E_im, E_re, R_im, R_re, mask, dtype="bf16"):
    """Host-side sharding / layout prep."""
    import ml_dtypes

    np_bf = ml_dtypes.bfloat16
    h32 = np.ascontiguousarray(np.asarray(h, dtype=np.int32).reshape(NB, 128).T)
    r32 = np.ascontiguousarray(np.asarray(r, dtype=np.int32).reshape(NB, 128).T)
    # idx[p, s] = s*16 + (p % 16), replicated to all 128 partitions (each
    # GpSimd core reads its own 16-partition group)
    gidx_np = np.ascontiguousarray(
        (
            np.arange(KCH, dtype=np.int16)[None, :] * 16
            + (np.arange(128, dtype=np.int16) % 16)[:, None]
        )
    )

    E_im = np.asarray(E_im, dtype=np.float32)
    E_re = np.asarray(E_re, dtype=np.float32)
    ec = np.concatenate([E_im, E_re], axis=1)
    ecb = np.ascontiguousarray(ec.astype(np_bf))

    R_im = np.asarray(R_im, dtype=np.float32)
    R_re = np.asarray(R_re, dtype=np.float32)
    ridx = np.arange(NUM_REL_TOTAL)
    reff = np.where(ridx >= NUM_REL, ridx - NUM_REL, ridx)
    sgn = np.where(ridx >= NUM_REL, -1.0, 1.0).astype(np.float32)[:, None]
    rpb = np.ascontiguousarray(
        np.concatenate([R_im[reff] * sgn, R_re[reff]], axis=1).astype(np_bf)
    )

    mask = np.asarray(mask, dtype=np.float32).reshape(1, NUM_ENTITIES)

    in_maps = []
    for k in range(N_CORES):
        sl = slice(k * ESH, (k + 1) * ESH)
        # ecatT[p, ((et*KCH)+c)*ET + j] = Ecat_k[et*ET + j, c*128 + p]
        ecat_k = ec[sl]  # [ESH, 1024] view
        ecatT = np.ascontiguousarray(
            ecat_k.reshape(N_ET, ET, KCH, 128)
            .transpose(3, 0, 2, 1)
            .reshape(128, N_ET * KCH * ET)
            .astype(np_bf)
        )
        in_maps.append(
            {
                "hix": h32,
                "rix": r32,
                "gidx": gidx_np,
                "ecb": ecb,
                "rpb": rpb,
                "ecatT": ecatT,
                "maskb": np.ascontiguousarray(
                    np.broadcast_to(mask[:, sl].astype(np_bf), (128, ESH))
                ),
            }
        )
    return in_maps


def kernel(h, r, E_im, E_re, R_im, R_re, mask):
    nc = _get_module()
    in_maps = make_in_maps(h, r, E_im, E_re, R_im, R_re, mask)
    res = run_bass_kernel_spmd(nc, in_maps, core_ids=list(range(N_CORES)))
    return np.concatenate(
        [res.results[k]["out"].astype(np.float32) for k in range(N_CORES)], axis=1
    )
